# revision 1
# baseline (speedup 1.0000x reference)
"""Multi-head causal attention (B=2, S=2048, D=2048, H=16) on 8 trn2 cores.

Sharding: tensor-parallel over heads. Core c owns heads 2c, 2c+1 (256
features of q/k/v). Each core computes its heads' QKV projections (+RoPE),
causal attention, and a partial output through its slice of wo. The 8
partial outputs are summed on the host (the "all-reduce").

Layouts on device (per core):
  qT, kT: [hd=128 partitions, head, token]  (features on partitions, f32r)
          head dims permuted (evens then odds) via host-permuted wq/wk rows
          so RoPE pairs sit in partition halves.
  v:      computed transposed like q/k (all QKV matmuls N=512, balanced
          LDWEIGHTS), then PE-transposed into natural [token, feature]
          blocks one pipeline step before attention consumes them.
  scoresT chunk = matmul(lhsT=kT chunk, rhs=qT tile) -> [kt 128, q 512] PSUM
  probsT = exp(scoresT/sqrt(hd)) (no max subtraction: scores are O(1)-bounded)
  attnT accum = matmul(lhsT=v chunk, rhs=probsT) -> [hd, q] PSUM
  denom = matmul(lhsT=ones, rhs=probsT) -> [1, q] PSUM; normalize at eviction.
  out rows = matmul(lhsT=attnT t-sub, rhs=woT) -> [t 128, j] PSUM -> DRAM.

RoPE (per psum eviction tile qp [128, 512], top rows = even dims xr,
bottom = odd dims xi):
  qswap = [xi; xr]        (2 ACT half-copies, psum base-shifted)
  p1 = qp * [c; c]        (DVE, psum x sbuf)
  p2 = qswap * [s; -s]    (DVE, sbuf 2x)
  qT  = p1 - p2 = [xr*c - xi*s ; xi*c + xr*s]   (DVE -> f32r)
"""

import math

import numpy as np

B = 2
S = 2048
D = 2048
H = 16
HD = 128
NCORES = 8
FPC = D // NCORES          # 256 features (2 heads) per core
P = 128
ND = D // P                # 16 contraction chunks
TT_N = 512                 # token tile (matmul free dim)
NTT = S // TT_N            # 4 token tiles per batch
NKT = S // P               # 16 key chunks per batch
SCALE = 1.0 / math.sqrt(HD)

_CACHE = {}


def _build_nc():
    import concourse.bass as bass  # noqa: F401
    from concourse import bacc
    import concourse.mybir as mybir
    import concourse.tile as tile

    f32 = mybir.dt.float32
    f32r = mybir.dt.float32r
    MUL = mybir.AluOpType.mult
    SUB = mybir.AluOpType.subtract

    nc = bacc.Bacc(None, target_bir_lowering=False)

    xT = nc.dram_tensor("xT", [D, B * S], f32r, kind="ExternalInput")
    wqT = nc.dram_tensor("wqT", [D, FPC], f32r, kind="ExternalInput")
    wkT = nc.dram_tensor("wkT", [D, FPC], f32r, kind="ExternalInput")
    wvT = nc.dram_tensor("wvT", [D, FPC], f32r, kind="ExternalInput")
    woT = nc.dram_tensor("woT", [FPC, D], f32r, kind="ExternalInput")
    cosS = nc.dram_tensor("cosS", [P, S], f32, kind="ExternalInput")
    sinS = nc.dram_tensor("sinS", [P, S], f32, kind="ExternalInput")  # [s; -s]
    masks = nc.dram_tensor("masks", [P, P], f32r, kind="ExternalInput")
    onesd = nc.dram_tensor("onesd", [P, 1], f32r, kind="ExternalInput")
    identd = nc.dram_tensor("identd", [P, P], f32r, kind="ExternalInput")
    outp = nc.dram_tensor("outp", [B * S, D], f32, kind="ExternalOutput")

    with tile.TileContext(nc) as tc:
        with (
            tc.tile_pool(name="res", bufs=1) as res,
            tc.tile_pool(name="xp", bufs=8) as xp,
            tc.tile_pool(name="aTp", bufs=2) as aTp,
            tc.tile_pool(name="probsp", bufs=8) as probsp,
            tc.tile_pool(name="ropep", bufs=2) as ropep,
            tc.tile_pool(name="csp", bufs=1) as csp,
            tc.tile_pool(name="recipp", bufs=2) as recipp,
            tc.tile_pool(name="rbp", bufs=2) as rbp,
            tc.tile_pool(name="ostp", bufs=3) as ostp,
            tc.tile_pool(name="vstp", bufs=2) as vstp,
            tc.tile_pool(name="ps_big", bufs=4, space="PSUM") as ps_big,
            tc.tile_pool(name="ps_av", bufs=2, space="PSUM") as ps_av,
            tc.tile_pool(name="ps_d", bufs=2, space="PSUM") as ps_d,
        ):
            # resident tensors
            wq_sb = res.tile([P, ND, FPC], f32r)
            wk_sb = res.tile([P, ND, FPC], f32r)
            wv_sb = res.tile([P, ND, FPC], f32r)
            wo_sb = res.tile([P, 2, D], f32r)
            mask_sb = res.tile([P, P], f32r)
            ones_sb = res.tile([P, 1], f32r)
            qT_sb = res.tile([P, 2, S], f32r)
            kT_sb = res.tile([P, 2, S], f32r)
            v_sb = res.tile([P, NKT, FPC], f32r)
            ident = res.tile([P, P], f32r)

            def load_w_chunk(d):
                nc.sync.dma_start(out=wq_sb[:, d, :], in_=wqT[d * P:(d + 1) * P, :])
                nc.sync.dma_start(out=wk_sb[:, d, :], in_=wkT[d * P:(d + 1) * P, :])
                nc.sync.dma_start(out=wv_sb[:, d, :], in_=wvT[d * P:(d + 1) * P, :])

            nc.scalar.dma_start(out=ones_sb[:], in_=onesd[:])
            nc.scalar.dma_start(out=ident[:], in_=identd[:])
            nc.scalar.dma_start(out=mask_sb[:], in_=masks[:])

            wo_jobs = []
            pending_vt = []

            def flush_vt():
                while pending_vt:
                    vst, tt = pending_vt.pop(0)
                    for sub in range(4):
                        for fc in range(2):
                            tr = ps_d.tile([P, P], f32r, name="dp")
                            nc.tensor.transpose(
                                tr[:], vst[:, fc, sub * P:(sub + 1) * P],
                                ident[:])
                            nc.scalar.copy(
                                v_sb[:, tt * 4 + sub, fc * P:(fc + 1) * P],
                                tr[:])

            def emit_wo_group(trow0, aT, ts, jc):
                trow = trow0 + ts * P
                o_ps = ps_av.tile([P, TT_N], f32, name="av")
                for h in range(2):
                    nc.tensor.matmul(
                        o_ps[:],
                        aT[:, h, ts * P:(ts + 1) * P],
                        wo_sb[:, h, jc * TT_N:(jc + 1) * TT_N],
                        start=(h == 0), stop=(h == 1),
                    )
                ost = ostp.tile([P, TT_N], f32, name="ost")
                if jc % 2 == 0:
                    nc.vector.tensor_copy(ost[:], o_ps[:])
                    nc.gpsimd.dma_start(
                        out=outp[trow:trow + P, jc * TT_N:(jc + 1) * TT_N],
                        in_=ost[:],
                    )
                else:
                    nc.scalar.copy(ost[:], o_ps[:])
                    nc.sync.dma_start(
                        out=outp[trow:trow + P, jc * TT_N:(jc + 1) * TT_N],
                        in_=ost[:],
                    )

            def emit_qkv(b, tt):
                t0g = b * S
                tsl = slice(tt * TT_N, (tt + 1) * TT_N)
                gsl = slice(t0g + tt * TT_N, t0g + (tt + 1) * TT_N)

                cct = csp.tile([P, TT_N], f32, name="cct")
                sst = csp.tile([P, TT_N], f32, name="sst")
                nc.sync.dma_start(out=cct[:], in_=cosS[:, tsl])
                nc.sync.dma_start(out=sst[:], in_=sinS[:, tsl])

                qk_ps = [ps_big.tile([P, TT_N], f32, name="big")
                         for _ in range(4)]
                vT_ps = [ps_av.tile([P, TT_N], f32, name="av")
                         for _ in range(2)]

                flush_vt()
                for d in range(ND):
                    xt = xp.tile([P, TT_N], f32r, name="xt")
                    xeng = nc.sync if d % 2 == 0 else nc.gpsimd
                    xeng.dma_start(out=xt[:], in_=xT[d * P:(d + 1) * P, gsl])
                    if b == 0 and tt == 0:
                        load_w_chunk(d)
                        if d == ND - 1:
                            for fc in range(2):
                                nc.sync.dma_start(
                                    out=wo_sb[:, fc, :],
                                    in_=woT[fc * P:(fc + 1) * P, :])
                    for fc in range(2):
                        nc.tensor.matmul(
                            qk_ps[fc][:],
                            wq_sb[:, d, fc * P:(fc + 1) * P],
                            xt[:],
                            start=(d == 0), stop=(d == ND - 1),
                        )
                        nc.tensor.matmul(
                            qk_ps[2 + fc][:],
                            wk_sb[:, d, fc * P:(fc + 1) * P],
                            xt[:],
                            start=(d == 0), stop=(d == ND - 1),
                        )
                    for fc in range(2):
                        nc.tensor.matmul(
                            vT_ps[fc][:],
                            wv_sb[:, d, fc * P:(fc + 1) * P],
                            xt[:],
                            start=(d == 0), stop=(d == ND - 1),
                        )
                vst = vstp.tile([P, 2, TT_N], f32r, name="vst")
                for fc in range(2):
                    nc.scalar.copy(vst[:, fc, :], vT_ps[fc][:])
                pending_vt.append((vst, tt))

                # RoPE + eviction for q,k
                for i, dst in ((0, qT_sb), (1, qT_sb), (2, kT_sb), (3, kT_sb)):
                    fc = i % 2
                    qp = qk_ps[i]
                    qsw = ropep.tile([P, TT_N], f32, name="qsw")
                    nc.scalar.copy(qsw[0:64, :], qp[64:128, :])
                    nc.scalar.copy(qsw[64:128, :], qp[0:64, :])
                    p1 = ropep.tile([P, TT_N], f32, name="p1")
                    nc.vector.tensor_tensor(
                        out=p1[:], in0=qp[:], in1=cct[:], op=MUL)
                    p2 = ropep.tile([P, TT_N], f32, name="p2")
                    nc.vector.tensor_tensor(
                        out=p2[:], in0=qsw[:], in1=sst[:], op=MUL)
                    nc.vector.tensor_tensor(
                        out=dst[:, fc, tsl], in0=p1[:], in1=p2[:], op=SUB)

            def emit_attention(b, qt):
                flush_vt()
                t0g = b * S
                nkt = 4 * qt + 4
                aT = aTp.tile([P, 2, TT_N], f32r, name="aT")
                niter = 2 * (nkt + 1)
                wo_per_iter = (len(wo_jobs) + niter - 1) // niter if wo_jobs else 0
                for h in range(2):
                    a_ps = ps_av.tile([P, TT_N], f32, name="av")
                    d_ps = ps_d.tile([1, TT_N], f32, name="dp")
                    prev = None
                    for kt in range(nkt + 1):
                        for _ in range(wo_per_iter):
                            if wo_jobs:
                                emit_wo_group(*wo_jobs.pop(0))
                        if kt < nkt:
                            o = kt - 4 * qt
                            c0 = max(o, 0) * P
                            csl = slice(c0, TT_N)
                            s_ps = ps_big.tile([P, TT_N], f32, name="big")
                            nc.tensor.matmul(
                                s_ps[:, csl],
                                kT_sb[:, h, kt * P:(kt + 1) * P],
                                qT_sb[:, h, qt * TT_N + c0:(qt + 1) * TT_N],
                                start=True, stop=True,
                            )
                            pr = probsp.tile([P, TT_N], f32r, name="probs")
                            nc.scalar.activation(
                                pr[:, csl], s_ps[:, csl],
                                mybir.ActivationFunctionType.Exp,
                                scale=SCALE,
                            )
                            if o >= 0:
                                nc.vector.tensor_tensor(
                                    out=pr[:, c0:c0 + P],
                                    in0=pr[:, c0:c0 + P],
                                    in1=mask_sb[:],
                                    op=MUL,
                                )
                            cur = (pr, c0)
                        if kt > 0:
                            pr, c0p = prev
                            psl = slice(c0p, TT_N)
                            nc.tensor.matmul(
                                a_ps[:, psl],
                                v_sb[:, kt - 1, h * P:(h + 1) * P],
                                pr[:, psl],
                                start=(kt == 1), stop=(kt == nkt),
                            )
                            nc.tensor.matmul(
                                d_ps[:, psl],
                                ones_sb[:],
                                pr[:, psl],
                                start=(kt == 1), stop=(kt == nkt),
                            )
                            if kt == nkt:
                                recip = recipp.tile([1, TT_N], f32, name="recip")
                                nc.vector.reciprocal(recip[:], d_ps[:])
                                rb = rbp.tile([P, TT_N], f32, name="rb")
                                nc.gpsimd.partition_broadcast(rb[:], recip[:])
                                nc.vector.tensor_tensor(
                                    out=aT[:, h, :], in0=a_ps[:],
                                    in1=rb[:], op=MUL)
                        if kt < nkt:
                            prev = cur
                for ts in range(TT_N // P):
                    for jc in range(D // TT_N):
                        wo_jobs.append((t0g + qt * TT_N, aT, ts, jc))

            # schedule: attention lags QKV by one tile within each batch
            for b in range(B):
                emit_qkv(b, 0)
                for tt in range(1, NTT):
                    emit_qkv(b, tt)
                    emit_attention(b, tt - 1)
                emit_attention(b, NTT - 1)
            while wo_jobs:
                emit_wo_group(*wo_jobs.pop(0))
    nc.compile()
    return nc


def _host_prep(x, wq, wk, wv, wo):
    x = np.asarray(x, dtype=np.float32)
    wq = np.asarray(wq, dtype=np.float32)
    wk = np.asarray(wk, dtype=np.float32)
    wv = np.asarray(wv, dtype=np.float32)
    wo = np.asarray(wo, dtype=np.float32)

    xT = np.ascontiguousarray(x.reshape(B * S, D).T)  # [D, B*S]

    # permute q/k head dims: per head, even dims then odd dims
    perm = np.concatenate(
        [h * HD + np.concatenate([np.arange(0, HD, 2), np.arange(1, HD, 2)])
         for h in range(H)]
    )
    wq_p = wq[perm]
    wk_p = wk[perm]

    # rope tables; cos stacked twice, sin stacked [s; -s]
    inv_freq = 1.0 / (10000.0 ** (np.arange(0, HD, 2, dtype=np.float64) / HD))
    t = np.arange(S, dtype=np.float64)
    freqs = t[:, None] * inv_freq[None, :]            # [S, 64]
    cosT = np.cos(freqs).T.astype(np.float32)         # [64, S]
    sinT = np.sin(freqs).T.astype(np.float32)
    cosS = np.ascontiguousarray(np.vstack([cosT, cosT]))
    sinS = np.ascontiguousarray(np.vstack([sinT, -sinT]))

    # triangular causal mask for the diagonal 128x128 block
    pidx = np.arange(P)[:, None]
    qidx = np.arange(P)[None, :]
    m = np.ascontiguousarray((qidx >= pidx).astype(np.float32))

    ones = np.ones((P, 1), dtype=np.float32)

    in_maps = []
    for c in range(NCORES):
        fs = slice(c * FPC, (c + 1) * FPC)
        in_maps.append({
            "xT": xT,
            "wqT": np.ascontiguousarray(wq_p[fs].T),   # [D, 256]
            "wkT": np.ascontiguousarray(wk_p[fs].T),
            "wvT": np.ascontiguousarray(wv[fs].T),
            "woT": np.ascontiguousarray(wo[:, fs].T),  # [256, D]
            "cosS": cosS,
            "sinS": sinS,
            "masks": m,
            "onesd": ones,
            "identd": np.eye(P, dtype=np.float32),
        })
    return in_maps


def _run(inputs, trace=False):
    from concourse.bass_utils import run_bass_kernel_spmd

    if "nc" not in _CACHE:
        _CACHE["nc"] = _build_nc()
    nc = _CACHE["nc"]

    in_maps = _host_prep(
        inputs["x"], inputs["wq"], inputs["wk"], inputs["wv"], inputs["wo"]
    )
    res = run_bass_kernel_spmd(nc, in_maps, list(range(NCORES)), trace=trace)
    acc = None
    for c in range(NCORES):
        part = res.results[c]["outp"]
        acc = part.copy() if acc is None else acc + part
    out = acc.reshape(B, S, D).astype(np.float32)
    return out, res


def kernel(**inputs) -> np.ndarray:
    out, _ = _run(inputs, trace=False)
    return out



# revision 2
# speedup vs baseline: 1.5972x; 1.5972x over previous
"""Multi-head causal attention (B=2, S=2048, D=2048, H=16) on 8 trn2 cores.

Sharding: tensor-parallel over heads. Core c owns heads 2c, 2c+1 (256
features of q/k/v). Each core computes its heads' QKV projections (+RoPE),
causal attention, and a partial output through its slice of wo. The 8
partial outputs are summed on the host (the "all-reduce").

All matmul operands are bf16 (fp32 PSUM accumulate); IO tensors are bf16
(halves HBM traffic, enables FWL weight loads). f32r and bf16 both stream
1 cycle/row on the PE, so precision is the only trade (measured ~3e-3 rel).

Per-core layouts:
  qT, kT: [hd=128 partitions, head, token] bf16. Head dims permuted (evens
          then odds) via host-permuted wq/wk rows so RoPE pairs sit in
          partition halves.
  v:      natural [token, feature] bf16, computed directly with x chunks as
          the stationary operand (no PE transposes).
  scoresT chunk = matmul(lhsT=kT chunk, rhs=qT tile) -> [kt 128, q<=512] PSUM
  probsT  = exp(scoresT * scale) on ACT -> bf16 SBUF (scores O(1), no max)
  denom   = running bf16 chunk-sum on DVE, then gpsimd partition_all_reduce
            (broadcast to 128 partitions), reciprocal_approx_fast on DVE.
            (No PE matmuls or PSUM banks burned on the softmax denominator.)
  attnT accum = matmul(lhsT=v chunk, rhs=probsT) -> [hd, q] PSUM, then
            normalized into aT bf16 at eviction.
  out rows = matmul(lhsT=aT t-sub, rhs=woT) -> [t 128, j 512] PSUM, staged
            into a [128, 4, 2048] bf16 tile, one 2MB DMA per (b, qtile).

QKV runs as six sequential single-PSUM-bank passes per token tile
(q0,q1,k0,k1,v...), which keeps the whole kernel inside 8 PSUM banks:
2 (qkv ping-pong) + 2 (scores) + 2 (attn accum) + 2 (wo out).

RoPE per eviction tile qp [128, 512] (top rows even dims xr, bottom odd xi):
  qraw = copy(qp)->bf16  (ACT; frees the PSUM bank immediately)
  qsw  = [xi; xr]        (2 ACT half-copies)
  qT   = qraw*[c;c] - qsw*[s;-s]   (3 DVE bf16 ops)
"""

import math

import numpy as np

B = 2
S = 2048
D = 2048
H = 16
HD = 128
NCORES = 8
FPC = D // NCORES          # 256 features (2 heads) per core
P = 128
ND = D // P                # 16 contraction chunks
TT = 512                   # token tile (matmul free dim)
NTT = S // TT              # 4 token tiles per batch
NKT = S // P               # 16 key chunks per batch
SCALE = 1.0 / math.sqrt(HD)

_CACHE = {}


def _build_nc():
    import concourse.bass as bass  # noqa: F401
    from concourse import bacc
    import concourse.bass_isa as bass_isa
    import concourse.mybir as mybir
    import concourse.tile as tile

    f32 = mybir.dt.float32
    bf16 = mybir.dt.bfloat16
    MUL = mybir.AluOpType.mult
    SUB = mybir.AluOpType.subtract
    ADD = mybir.AluOpType.add
    EXP = mybir.ActivationFunctionType.Exp

    nc = bacc.Bacc(None, target_bir_lowering=False)

    xTb = nc.dram_tensor("xTb", [D, B * S], bf16, kind="ExternalInput")
    wqT = nc.dram_tensor("wqT", [D, FPC], bf16, kind="ExternalInput")
    wkT = nc.dram_tensor("wkT", [D, FPC], bf16, kind="ExternalInput")
    wvT = nc.dram_tensor("wvT", [D, FPC], bf16, kind="ExternalInput")
    woT = nc.dram_tensor("woT", [FPC, D], bf16, kind="ExternalInput")
    cosS = nc.dram_tensor("cosS", [P, S], bf16, kind="ExternalInput")
    sinS = nc.dram_tensor("sinS", [P, S], bf16, kind="ExternalInput")  # [s; -s]
    masks = nc.dram_tensor("masks", [P, 2 * P], bf16, kind="ExternalInput")
    outp = nc.dram_tensor("outp", [B * S, D], bf16, kind="ExternalOutput")

    xTr = xTb.rearrange("(n p) s -> p n s", p=P)       # [128, 16, B*S]
    outr = outp.rearrange("(r p) d -> p r d", p=P)     # [128, 32, D]

    with tile.TileContext(nc) as tc:
        with (
            tc.tile_pool(name="res", bufs=1) as res,
            tc.tile_pool(name="xp", bufs=2) as xp,
            tc.tile_pool(name="ropep", bufs=2) as ropep,
            tc.tile_pool(name="probsp", bufs=6) as probsp,
            tc.tile_pool(name="accp", bufs=2) as accp,
            tc.tile_pool(name="dp", bufs=2) as dp,
            tc.tile_pool(name="aTp", bufs=2) as aTp,
            tc.tile_pool(name="ostp", bufs=2) as ostp,
            tc.tile_pool(name="ps_qkv", bufs=2, space="PSUM") as ps_qkv,
            tc.tile_pool(name="ps_s", bufs=2, space="PSUM") as ps_s,
            tc.tile_pool(name="ps_a", bufs=2, space="PSUM") as ps_a,
            tc.tile_pool(name="ps_o", bufs=2, space="PSUM") as ps_o,
        ):
            # resident tensors
            wq_sb = res.tile([P, ND, FPC], bf16)
            wk_sb = res.tile([P, ND, FPC], bf16)
            wv_sb = res.tile([P, ND, FPC], bf16)
            wo_sb = res.tile([P, 2, D], bf16)
            cos_sb = res.tile([P, S], bf16)
            sin_sb = res.tile([P, S], bf16)
            mask_sb = res.tile([P, 2 * P], bf16)
            qT_sb = res.tile([P, 2, S], bf16)
            kT_sb = res.tile([P, 2, S], bf16)
            v_sb = res.tile([P, NKT, FPC], bf16)

            # weight/constant loads on the SWDGE queue, ordered by first use;
            # x tiles + output stores ride the HWDGE (sync) queue.
            nc.gpsimd.dma_start(
                out=wq_sb[:], in_=wqT.rearrange("(n p) f -> p n f", p=P))
            nc.gpsimd.dma_start(out=cos_sb[:], in_=cosS[:])
            nc.gpsimd.dma_start(out=sin_sb[:], in_=sinS[:])
            nc.gpsimd.dma_start(out=mask_sb[:], in_=masks[:])
            nc.gpsimd.dma_start(
                out=wk_sb[:], in_=wkT.rearrange("(n p) f -> p n f", p=P))
            nc.gpsimd.dma_start(
                out=wv_sb[:], in_=wvT.rearrange("(n p) f -> p n f", p=P))
            nc.gpsimd.dma_start(
                out=wo_sb[:], in_=woT.rearrange("(c p) d -> p c d", p=P))

            wo_jobs = []
            ost_state = {}  # id(ost) -> [count, b, qt]

            def emit_wo_group(b, qt, aT, ts, jc, ost):
                o_ps = ps_o.tile([P, TT], f32, name="ops")
                for h in range(2):
                    nc.tensor.matmul(
                        o_ps[:],
                        aT[:, h, ts * P:(ts + 1) * P],
                        wo_sb[:, h, jc * TT:(jc + 1) * TT],
                        start=(h == 0), stop=(h == 1),
                    )
                nc.any.tensor_copy(ost[:, ts, jc * TT:(jc + 1) * TT], o_ps[:])
                st = ost_state[id(ost)]
                st[0] += 1
                if st[0] == 16:
                    r0 = (b * S + qt * TT) // P
                    nc.sync.dma_start(out=outr[:, r0:r0 + 4, :], in_=ost[:])

            def pop_wo(n=1):
                for _ in range(n):
                    if wo_jobs:
                        emit_wo_group(*wo_jobs.pop(0))

            def rope_evict(ps, dst, fc, tsl):
                qraw = ropep.tile([P, TT], bf16, name="qraw")
                nc.scalar.copy(qraw[:], ps[:])
                qsw = ropep.tile([P, TT], bf16, name="qsw")
                nc.scalar.copy(qsw[0:64, :], qraw[64:128, :])
                nc.scalar.copy(qsw[64:128, :], qraw[0:64, :])
                p1 = ropep.tile([P, TT], bf16, name="p1")
                nc.vector.tensor_tensor(
                    out=p1[:], in0=qraw[:], in1=cos_sb[:, tsl], op=MUL)
                p2 = ropep.tile([P, TT], bf16, name="p2")
                nc.vector.tensor_tensor(
                    out=p2[:], in0=qsw[:], in1=sin_sb[:, tsl], op=MUL)
                nc.vector.tensor_tensor(
                    out=dst[:, fc, tsl], in0=p1[:], in1=p2[:], op=SUB)

            def emit_qkv(b, tt):
                t0g = b * S
                tsl = slice(tt * TT, (tt + 1) * TT)
                gsl = slice(t0g + tt * TT, t0g + (tt + 1) * TT)

                xt = xp.tile([P, ND, TT], bf16, name="xt")
                nc.sync.dma_start(out=xt[:], in_=xTr[:, :, gsl])

                # q/k passes: one PSUM bank per (which, fc), sequential
                for w_sb, dst in ((wq_sb, qT_sb), (wk_sb, kT_sb)):
                    for fc in range(2):
                        ps = ps_qkv.tile([P, TT], f32, name="qkv")
                        for d in range(ND):
                            nc.tensor.matmul(
                                ps[:],
                                w_sb[:, d, fc * P:(fc + 1) * P],
                                xt[:, d, :],
                                start=(d == 0), stop=(d == ND - 1),
                            )
                        rope_evict(ps, dst, fc, tsl)
                        pop_wo()
                # v pass: natural layout, one 128-token sub-chunk per bank
                for sub in range(4):
                    ps = ps_qkv.tile([P, TT], f32, name="qkv")
                    for d in range(ND):
                        nc.tensor.matmul(
                            ps[:, 0:FPC],
                            xt[:, d, sub * P:(sub + 1) * P],
                            wv_sb[:, d, :],
                            start=(d == 0), stop=(d == ND - 1),
                        )
                    nc.any.tensor_copy(
                        v_sb[:, tt * 4 + sub, :], ps[:, 0:FPC])
                    pop_wo()

            def emit_attention(b, qt):
                t0g = b * S
                nkt = 4 * qt + 4
                aT = aTp.tile([P, 2, TT], bf16, name="aT")
                ost = ostp.tile([P, 4, D], bf16, name="ost")
                ost_state[id(ost)] = [0, b, qt]
                # spread pending wo jobs over this tile's chunk iterations
                niter = 2 * nkt
                wo_quota = len(wo_jobs)
                emitted = [0]

                def pace(i):
                    want = wo_quota * (i + 1) // niter
                    while emitted[0] < want and wo_jobs:
                        emit_wo_group(*wo_jobs.pop(0))
                        emitted[0] += 1

                it = 0
                for h in range(2):
                    acc = accp.tile([P, TT], bf16, name="acc")
                    a_ps = ps_a.tile([P, TT], f32, name="aps")
                    for kt in range(nkt):
                        pace(it); it += 1
                        o = kt - 4 * qt
                        c0 = 0 if o < 0 else min(o * P, 2 * P)
                        csl = slice(c0, TT)
                        s_ps = ps_s.tile([P, TT], f32, name="sps")
                        nc.tensor.matmul(
                            s_ps[:, csl],
                            kT_sb[:, h, kt * P:(kt + 1) * P],
                            qT_sb[:, h, qt * TT + c0:(qt + 1) * TT],
                            start=True, stop=True,
                        )
                        pr = probsp.tile([P, TT], bf16, name="probs")
                        nc.scalar.activation(
                            pr[:, csl], s_ps[:, csl], EXP, scale=SCALE)
                        if o == 3:
                            nc.vector.tensor_tensor(
                                out=pr[:, 2 * P:TT], in0=pr[:, 2 * P:TT],
                                in1=mask_sb[:], op=MUL)
                        elif o >= 0:
                            nc.vector.tensor_tensor(
                                out=pr[:, c0:c0 + P], in0=pr[:, c0:c0 + P],
                                in1=mask_sb[:, P:2 * P], op=MUL)
                        if kt == 0:
                            nc.vector.tensor_copy(acc[:], pr[:])
                        else:
                            nc.vector.tensor_tensor(
                                out=acc[:, csl], in0=acc[:, csl],
                                in1=pr[:, csl], op=ADD)
                        nc.tensor.matmul(
                            a_ps[:, csl],
                            v_sb[:, kt, h * P:(h + 1) * P],
                            pr[:, csl],
                            start=(kt == 0), stop=(kt == nkt - 1),
                        )
                    den = dp.tile([P, TT], f32, name="den")
                    nc.gpsimd.partition_all_reduce(
                        den[:], acc[:], channels=P,
                        reduce_op=bass_isa.ReduceOp.add)
                    rb = dp.tile([P, TT], f32, name="rb")
                    nc.vector.reciprocal_approx_fast(rb[:], den[:])
                    nc.vector.tensor_tensor(
                        out=aT[:, h, :], in0=a_ps[:], in1=rb[:], op=MUL)
                for ts in range(4):
                    for jc in range(D // TT):
                        wo_jobs.append((b, qt, aT, ts, jc, ost))

            # schedule: attention lags QKV by one tile within each batch
            for b in range(B):
                emit_qkv(b, 0)
                for tt in range(1, NTT):
                    emit_qkv(b, tt)
                    emit_attention(b, tt - 1)
                emit_attention(b, NTT - 1)
            pop_wo(len(wo_jobs))
    nc.compile()
    return nc


def _host_prep(x, wq, wk, wv, wo):
    import ml_dtypes

    bf = ml_dtypes.bfloat16
    x = np.asarray(x, dtype=np.float32)
    wq = np.asarray(wq, dtype=np.float32)
    wk = np.asarray(wk, dtype=np.float32)
    wv = np.asarray(wv, dtype=np.float32)
    wo = np.asarray(wo, dtype=np.float32)

    xT = np.ascontiguousarray(x.reshape(B * S, D).T).astype(bf)  # [D, B*S]

    # permute q/k head dims: per head, even dims then odd dims
    perm = np.concatenate(
        [h * HD + np.concatenate([np.arange(0, HD, 2), np.arange(1, HD, 2)])
         for h in range(H)]
    )
    wq_p = wq[perm]
    wk_p = wk[perm]

    # rope tables; cos stacked twice, sin stacked [s; -s]
    inv_freq = 1.0 / (10000.0 ** (np.arange(0, HD, 2, dtype=np.float64) / HD))
    t = np.arange(S, dtype=np.float64)
    freqs = t[:, None] * inv_freq[None, :]            # [S, 64]
    cosT = np.cos(freqs).T.astype(np.float32)         # [64, S]
    sinT = np.sin(freqs).T.astype(np.float32)
    cosS = np.ascontiguousarray(np.vstack([cosT, cosT])).astype(bf)
    sinS = np.ascontiguousarray(np.vstack([sinT, -sinT])).astype(bf)

    # masks: [zeros(128) | lower-triangular(128)] for the diagonal blocks
    pidx = np.arange(P)[:, None]
    qidx = np.arange(P)[None, :]
    tri = (qidx >= pidx).astype(np.float32)
    m = np.ascontiguousarray(
        np.hstack([np.zeros((P, P), np.float32), tri])).astype(bf)

    in_maps = []
    for c in range(NCORES):
        fs = slice(c * FPC, (c + 1) * FPC)
        in_maps.append({
            "xTb": xT,
            "wqT": np.ascontiguousarray(wq_p[fs].T).astype(bf),   # [D, 256]
            "wkT": np.ascontiguousarray(wk_p[fs].T).astype(bf),
            "wvT": np.ascontiguousarray(wv[fs].T).astype(bf),
            "woT": np.ascontiguousarray(wo[:, fs].T).astype(bf),  # [256, D]
            "cosS": cosS,
            "sinS": sinS,
            "masks": m,
        })
    return in_maps


def _run(inputs, trace=False):
    from concourse.bass_utils import run_bass_kernel_spmd

    if "nc" not in _CACHE:
        _CACHE["nc"] = _build_nc()
    nc = _CACHE["nc"]

    in_maps = _host_prep(
        inputs["x"], inputs["wq"], inputs["wk"], inputs["wv"], inputs["wo"]
    )
    res = run_bass_kernel_spmd(nc, in_maps, list(range(NCORES)), trace=trace)
    acc = None
    for c in range(NCORES):
        part = res.results[c]["outp"].astype(np.float32)
        acc = part if acc is None else acc + part
    out = acc.reshape(B, S, D).astype(np.float32)
    return out, res


def kernel(**inputs) -> np.ndarray:
    out, _ = _run(inputs, trace=False)
    return out
